# revision 16
# baseline (speedup 1.0000x reference)
"""BayesianLinear forward on 8 Trainium2 NeuronCores.

y = x @ W^T + b with W = w_mu + softplus(w_rho) * eps_w,
                     b = b_mu + softplus(b_rho) * eps_b.

Sharding: column-parallel (output features / 8). Each core samples its
weight shard on-chip and computes y^T[o_shard, :] = W_shard @ x^T.

Layout trick: everything is fed pre-transposed from the host so that the
contraction dim (in_features) lands on SBUF partitions with natural,
contiguous DMA patterns. The kernel emits y^T; the host transposes back.
"""

import numpy as np

# Problem shape (hardcoded per contest rules; kernel.py must be self-contained).
IN_F = 4096
OUT_F = 4096
N_TOK = 4096
N_CORES = 8
O_SHARD = OUT_F // N_CORES  # 512 output features per core

P = 128                     # SBUF partitions
KT = IN_F // P              # 32 contraction tiles
MS = O_SHARD // P           # 4 output-feature subtiles per core
N_TILE = 512                # moving-operand tile (fp32 PSUM bank limit)
NT = N_TOK // N_TILE        # 8 token tiles

_CACHE = {}


def _build_nc():
    import concourse.bass as bass  # noqa: F401
    from concourse import bacc, mybir
    from concourse.tile import TileContext

    f32 = mybir.dt.float32
    f32r = mybir.dt.float32r
    AF = mybir.ActivationFunctionType

    nc = bacc.Bacc("TRN2", target_bir_lowering=False, debug=False,
                   num_devices=N_CORES)

    x_t = nc.dram_tensor("x_t", [IN_F, N_TOK], f32r, kind="ExternalInput")
    w_mu_t = nc.dram_tensor("w_mu_t", [IN_F, O_SHARD], f32, kind="ExternalInput")
    w_rho_t = nc.dram_tensor("w_rho_t", [IN_F, O_SHARD], f32, kind="ExternalInput")
    eps_w_t = nc.dram_tensor("eps_w_t", [IN_F, O_SHARD], f32, kind="ExternalInput")
    b_mu = nc.dram_tensor("b_mu", [O_SHARD], f32, kind="ExternalInput")
    b_rho = nc.dram_tensor("b_rho", [O_SHARD], f32, kind="ExternalInput")
    eps_b = nc.dram_tensor("eps_b", [O_SHARD], f32, kind="ExternalInput")
    y_t = nc.dram_tensor("y_t", [O_SHARD, N_TOK], f32, kind="ExternalOutput")

    # k-tiles sampled per chunk (batched ACT/DVE). A small first chunk gets
    # the PE going early; later chunks amortize ACT instruction overhead.
    CHUNKS = [2, 6, 8, 8, 8]
    PRO_NT = 2              # token tiles computed during the prologue

    with TileContext(nc) as tc:
        with (
            tc.tile_pool(name="wpool", bufs=1) as wpool,
            tc.tile_pool(name="ppool", bufs=2) as ppool,
            tc.tile_pool(name="spool", bufs=1) as spool,
            tc.tile_pool(name="bpool", bufs=1) as bpool,
            tc.tile_pool(name="xpool", bufs=20) as xpool,
            tc.tile_pool(name="opool", bufs=8) as opool,
            tc.tile_pool(name="psum", bufs=8, space="PSUM") as psum,
        ):
            # ---- bias vector: b = b_mu + softplus(b_rho) * eps_b ----
            # laid out [P, MS]: partition p of output subtile ms holds
            # b[ms*128 + p].
            bmu_sb = bpool.tile([P, MS], f32, tag="bmu")
            brho_sb = bpool.tile([P, MS], f32, tag="brho")
            beps_sb = bpool.tile([P, MS], f32, tag="beps")
            bvec = bpool.tile([P, MS], f32, tag="bvec")
            nc.sync.dma_start(bmu_sb[:], b_mu.rearrange("(s p) -> p s", p=P))
            nc.sync.dma_start(brho_sb[:], b_rho.rearrange("(s p) -> p s", p=P))
            nc.sync.dma_start(beps_sb[:], eps_b.rearrange("(s p) -> p s", p=P))
            # softplus(r) = ln(1 + exp(r)); Exp and Ln share one ACT table.
            nc.scalar.activation(bvec[:], brho_sb[:], AF.Exp)
            nc.vector.tensor_scalar_add(bvec[:], bvec[:], 1.0)
            nc.scalar.activation(bvec[:], bvec[:], AF.Ln)
            nc.vector.tensor_mul(bvec[:], bvec[:], beps_sb[:])
            nc.vector.tensor_add(bvec[:], bvec[:], bmu_sb[:])

            # W^T shard stays resident in SBUF; float32r so the sampling's
            # final DVE add rounds W to the PE's fast-fp32 format.
            w_sb = wpool.tile([P, KT, O_SHARD], f32r, tag="w")
            # x^T column slices, rounded to fp32r on the host.
            x_tiles = {}

            def load_x(nt, kt):
                xt = xpool.tile([P, N_TILE], f32r, tag="x",
                                name=f"xt_{nt}_{kt}")
                nc.sync.dma_start(
                    xt[:], x_t[kt * P:(kt + 1) * P,
                               nt * N_TILE:(nt + 1) * N_TILE])
                x_tiles[(nt, kt)] = xt

            def mms(nt, psums, kt):
                xt = x_tiles.pop((nt, kt))
                for ms in range(MS):
                    nc.tensor.matmul(
                        psums[ms][:],
                        lhsT=w_sb[:, kt, ms * P:(ms + 1) * P],
                        rhs=xt[:],
                        start=(kt == 0),
                        stop=(kt == KT - 1),
                    )

            def store(nt, psums):
                nsl = slice(nt * N_TILE, (nt + 1) * N_TILE)
                for ms in range(MS):
                    ot = opool.tile([P, N_TILE], f32, tag="o",
                                    name=f"ot_{nt}_{ms}")
                    nc.vector.tensor_scalar_add(ot[:], psums[ms][:],
                                                bvec[:, ms:ms + 1])
                    nc.sync.dma_start(y_t[ms * P:(ms + 1) * P, nsl], ot[:])

            def alloc_psums(nt):
                return [psum.tile([P, N_TILE], f32, tag="ps",
                                  name=f"ps_{nt}_{i}") for i in range(MS)]

            # ---- prologue: sample W chunk-wise, overlapped with the first
            # PRO_NT token tiles' matmuls (8 PSUM banks = 2 nt x 4 ms) ----
            pro_psums = [alloc_psums(nt) for nt in range(PRO_NT)]
            kt0 = 0
            for c, CH in enumerate(CHUNKS):
                rows = slice(kt0 * P, (kt0 + CH) * P)
                csl = slice(kt0, kt0 + CH)
                s = spool.tile([P, CH, O_SHARD], f32, tag="s", name=f"s_{c}",
                               padded_shape=[P, max(CHUNKS), O_SHARD])
                mu = ppool.tile([P, CH, O_SHARD], f32, tag="mu", name=f"mu_{c}",
                                padded_shape=[P, max(CHUNKS), O_SHARD])
                eps = ppool.tile([P, CH, O_SHARD], f32, tag="eps",
                                 name=f"eps_{c}",
                                 padded_shape=[P, max(CHUNKS), O_SHARD])
                nc.sync.dma_start(
                    s[:], w_rho_t[rows, :].rearrange("(j p) o -> p j o", p=P))
                nc.sync.dma_start(
                    mu[:], w_mu_t[rows, :].rearrange("(j p) o -> p j o", p=P))
                nc.sync.dma_start(
                    eps[:], eps_w_t[rows, :].rearrange("(j p) o -> p j o", p=P))
                nc.scalar.activation(s[:], s[:], AF.Exp)
                nc.vector.tensor_scalar_add(s[:], s[:], 1.0)
                nc.scalar.activation(s[:], s[:], AF.Ln)
                nc.vector.tensor_mul(s[:], s[:], eps[:])
                nc.vector.tensor_add(w_sb[:, csl, :], s[:], mu[:])
                for j in range(CH):
                    for nt in range(PRO_NT):
                        load_x(nt, kt0 + j)
                for j in range(CH):
                    for nt in range(PRO_NT):
                        mms(nt, pro_psums[nt], kt0 + j)
                kt0 += CH
            for nt in range(PRO_NT):
                store(nt, pro_psums[nt])

            # ---- steady state: one token tile at a time ----
            for nt in range(PRO_NT, NT):
                for kt in range(KT):
                    load_x(nt, kt)
                psums = alloc_psums(nt)
                for kt in range(KT):
                    mms(nt, psums, kt)
                store(nt, psums)

    nc.compile()
    return nc


def _get_nc():
    if "nc" not in _CACHE:
        _CACHE["nc"] = _build_nc()
    return _CACHE["nc"]


def _round_fp32r(a):
    """Round-to-nearest-even into the PE's fp32r format (1s+8e+11m in the
    top 20 bits of the f32 word); the BIR verifier requires fp32r matmul
    operands to be pre-rounded."""
    u = a.view(np.uint32)
    r = (u + np.uint32(0x7FF) + ((u >> np.uint32(12)) & np.uint32(1))) \
        & np.uint32(0xFFFFF000)
    return r.view(np.float32)


def _in_maps(inputs):
    x = np.ascontiguousarray(np.asarray(inputs["x"], dtype=np.float32))
    w_mu = np.asarray(inputs["w_mu"], dtype=np.float32)
    w_rho = np.asarray(inputs["w_rho"], dtype=np.float32)
    eps_w = np.asarray(inputs["eps_w"], dtype=np.float32)
    b_mu = np.asarray(inputs["b_mu"], dtype=np.float32)
    b_rho = np.asarray(inputs["b_rho"], dtype=np.float32)
    eps_b = np.asarray(inputs["eps_b"], dtype=np.float32)

    x_t = _round_fp32r(np.ascontiguousarray(x.T))
    maps = []
    for c in range(N_CORES):
        sl = slice(c * O_SHARD, (c + 1) * O_SHARD)
        maps.append({
            "x_t": x_t,
            "w_mu_t": np.ascontiguousarray(w_mu[sl].T),
            "w_rho_t": np.ascontiguousarray(w_rho[sl].T),
            "eps_w_t": np.ascontiguousarray(eps_w[sl].T),
            "b_mu": np.ascontiguousarray(b_mu[sl]),
            "b_rho": np.ascontiguousarray(b_rho[sl]),
            "eps_b": np.ascontiguousarray(eps_b[sl]),
        })
    return maps


def run(inputs, trace=False, **kwargs):
    """Run on hardware; returns (y [N_TOK, OUT_F], BassKernelResults)."""
    from concourse.bass_utils import run_bass_kernel_spmd

    nc = _get_nc()
    res = run_bass_kernel_spmd(nc, _in_maps(inputs), list(range(N_CORES)),
                               trace=trace, **kwargs)
    y_t = np.concatenate([r["y_t"] for r in res.results], axis=0)
    return np.ascontiguousarray(y_t.T), res


def kernel(**inputs) -> np.ndarray:
    y, _ = run(inputs, trace=False)
    return y
